# revision 1
# baseline (speedup 1.0000x reference)
"""MoE (top-2, 8 experts) expert-parallel kernel for 8 Trainium2 NeuronCores.

Strategy
--------
The reference pads every expert to capacity N/K = 8192 token slots, but only
~N*K/E ~ 4096 per expert carry nonzero gate weight.  The router and the
dispatch/combine bookkeeping are tiny (<0.05% of FLOPs) and routing decisions
must match the reference bit-for-bit, so they are computed with the same jax
ops on CPU (the reference itself can only run on CPU: jnp.argsort has no
trn2 lowering).  Each expert's kept tokens are packed densely and shipped to
one NeuronCore: core e holds expert e's weights in SBUF and runs
    out = (relu(xg @ W1 + b1) @ W2 + b2) * gate
in bf16 (fp32 PSUM accumulation), streaming 512-token chunks.  The host then
scatter-adds the two expert contributions per token (exact: fp addition of
two terms is commutative) and returns (final, aux_loss).
"""

import math

import numpy as np

import concourse.bass as bass
import concourse.mybir as mybir
import concourse.tile as tile
from concourse.bass_utils import run_bass_kernel_spmd

dt = mybir.dt
BF16 = mybir.dt.np(mybir.dt.bfloat16)

N_CORES = 8
CHUNK = 512

# Filled by kernel() for test harness introspection / re-benching.
LAST = {}


def _split_multiwaits(nc):
    """The walrus build in this container accepts only ONE sem-wait per
    instruction (CoreV3 setupSyncWait: 'Too many sync wait commands').
    Tile attaches several waits to its end-of-kernel drain; move every
    extra wait onto a NoOp on the same engine immediately before the
    instruction (semantically identical: all waits must pass before the
    instruction may issue)."""
    uid = 0
    for fn in nc.m.functions:
        for bb in fn.blocks:
            out = []
            changed = False
            for inst in bb.instructions:
                si = inst.sync_info
                if si is not None and len(si.on_wait) > 1:
                    waits = list(si.on_wait)
                    for w in waits[:-1]:
                        uid += 1
                        out.append(
                            mybir.InstNoOp(
                                name=f"I-mwsplit-{uid}",
                                engine=inst.engine,
                                ins=[],
                                outs=[],
                                sync_info=mybir.SyncInfo(on_wait=[w], on_update=[]),
                            )
                        )
                    si.on_wait = [waits[-1]]
                    changed = True
                out.append(inst)
            if changed:
                bb.instructions = out


_PROG_CACHE = {}


def _build_program(T, D, H):
    """Per-core expert MLP: xT [128, D/128, T] bf16 -> outT [128, D/128, T] f32.

    Feature-major layout keeps tokens on the free dim, so b1/b2 are
    per-partition ACT biases and the per-token gate is a replicated
    [128, T] operand for one DVE multiply."""
    key = (T, D, H)
    if key in _PROG_CACHE:
        return _PROG_CACHE[key]

    KD = D // 128  # 8
    KH = H // 128  # 32
    n_chunks = T // CHUNK

    nc = bass.Bass()
    xT_d = nc.dram_tensor("xT", [128, KD, T], dt.bfloat16, kind="ExternalInput")
    w1_d = nc.dram_tensor("w1", [128, KD, H], dt.bfloat16, kind="ExternalInput")
    w2_d = nc.dram_tensor("w2", [128, KH, D], dt.bfloat16, kind="ExternalInput")
    b1_d = nc.dram_tensor("b1t", [128, KH], dt.float32, kind="ExternalInput")
    b2_d = nc.dram_tensor("b2t", [128, KD], dt.float32, kind="ExternalInput")
    wg_d = nc.dram_tensor("wg", [128, T], dt.float32, kind="ExternalInput")
    out_d = nc.dram_tensor("outT", [128, KD, T], dt.float32, kind="ExternalOutput")

    with tile.TileContext(nc) as tc:
        with (
            tc.tile_pool(name="wpool", bufs=1) as wpool,
            tc.tile_pool(name="hpool", bufs=1) as hpool,
            tc.tile_pool(name="xpool", bufs=2) as xpool,
            tc.tile_pool(name="gpool", bufs=2) as gpool,
            tc.tile_pool(name="opool", bufs=3) as opool,
            tc.tile_pool(name="ps1", bufs=3, space="PSUM") as ps1,
            tc.tile_pool(name="ps2", bufs=3, space="PSUM") as ps2,
        ):
            w1_s = wpool.tile([128, KD, H], dt.bfloat16)
            for kd in range(KD):
                nc.sync.dma_start(w1_s[:, kd], w1_d[:, kd])
            w2_s = wpool.tile([128, KH, D], dt.bfloat16)
            for kh in range(0, KH, 4):
                nc.sync.dma_start(w2_s[:, kh : kh + 4], w2_d[:, kh : kh + 4])
            b1_s = wpool.tile([128, KH], dt.float32)
            nc.sync.dma_start(b1_s[:], b1_d[:])
            b2_s = wpool.tile([128, KD], dt.float32)
            nc.sync.dma_start(b2_s[:], b2_d[:])

            hT = hpool.tile([128, KH, CHUNK], dt.bfloat16)

            for c in range(n_chunks):
                cs = bass.ds(c * CHUNK, CHUNK)
                xc = xpool.tile([128, KD, CHUNK], dt.bfloat16)
                nc.sync.dma_start(xc[:], xT_d[:, :, cs])
                gc = gpool.tile([128, CHUNK], dt.float32)
                nc.sync.dma_start(gc[:], wg_d[:, cs])

                for mh in range(KH):
                    p1 = ps1.tile([128, CHUNK], dt.float32)
                    for kd in range(KD):
                        nc.tensor.matmul(
                            p1[:],
                            w1_s[:, kd, bass.ts(mh, 128)],
                            xc[:, kd],
                            start=(kd == 0),
                            stop=(kd == KD - 1),
                        )
                    nc.scalar.activation(
                        hT[:, mh],
                        p1[:],
                        mybir.ActivationFunctionType.Relu,
                        bias=b1_s[:, mh : mh + 1],
                    )

                for md in range(KD):
                    p2 = ps2.tile([128, CHUNK], dt.float32)
                    for kh in range(KH):
                        nc.tensor.matmul(
                            p2[:],
                            w2_s[:, kh, bass.ts(md, 128)],
                            hT[:, kh],
                            start=(kh == 0),
                            stop=(kh == KH - 1),
                        )
                    ot = opool.tile([128, CHUNK], dt.float32)
                    nc.scalar.activation(
                        ot[:],
                        p2[:],
                        mybir.ActivationFunctionType.Identity,
                        bias=b2_s[:, md : md + 1],
                    )
                    nc.vector.tensor_mul(out=ot[:], in0=ot[:], in1=gc[:])
                    nc.sync.dma_start(out_d[:, md, cs], ot[:])

    _split_multiwaits(nc)
    _PROG_CACHE[key] = nc
    return nc


def _route(x, Wr, br, E, K):
    """Replicate the reference router bit-for-bit with the same jax ops on
    CPU (the backend the reference itself must run on)."""
    import jax
    import jax.numpy as jnp

    n_tok = x.shape[0]
    capacity = max(int(1.0 * n_tok / K), 1)  # CAPACITY_FACTOR=1, WORLD_SIZE=1

    cpu = jax.devices("cpu")[0]
    with jax.default_device(cpu):
        xj = jnp.asarray(x)
        Wrj = jnp.asarray(Wr)
        brj = jnp.asarray(br)

        logits = xj @ Wrj + brj
        probs = jax.nn.softmax(logits, axis=-1)
        ew, ei = jax.lax.top_k(probs, K)

        match = ei[:, :, None] == jnp.arange(E)[None, None, :]
        mask = match.any(axis=1)
        w = (ew[:, :, None] * match).sum(axis=1)

        pos = jnp.cumsum(mask.astype(jnp.int32), axis=0) - 1
        keep = mask & (pos < capacity)
        keepT = keep.T
        sel = jax.vmap(lambda k: jnp.argsort(jnp.logical_not(k))[:capacity])(keepT)

        expert_counts = mask.astype(probs.dtype).sum(axis=0)
        density = probs.mean(axis=0)
        usage = expert_counts / n_tok
        balance_loss = (density * usage).sum() * E
        importance = probs.sum(axis=0)
        importance_loss = (importance**2).mean()
        aux_loss = balance_loss + importance_loss

    return (
        np.asarray(sel),
        np.asarray(keep),
        np.asarray(pos),
        np.asarray(ei),
        np.asarray(w),
        np.asarray(aux_loss),
    )


def kernel(x, Wr, br, W1, b1, W2, b2):
    x = np.asarray(x)
    Wr = np.asarray(Wr)
    br = np.asarray(br)
    W1 = np.asarray(W1)
    b1 = np.asarray(b1)
    W2 = np.asarray(W2)
    b2 = np.asarray(b2)

    N, D = x.shape
    E, _, H = W1.shape
    K = 2

    sel, keep, pos, ei, w, aux_loss = _route(x, Wr, br, E, K)

    n_e = keep.sum(axis=0).astype(np.int64)  # kept tokens per expert
    T = max(CHUNK, int(math.ceil(n_e.max() / CHUNK)) * CHUNK)

    KD = D // 128
    KH = H // 128

    # Pack per-expert inputs (feature-major, bf16) + replicated gate rows.
    w1h = np.ascontiguousarray(
        W1.reshape(E, KD, 128, H).transpose(0, 2, 1, 3)
    ).astype(BF16)
    w2h = np.ascontiguousarray(
        W2.reshape(E, KH, 128, D).transpose(0, 2, 1, 3)
    ).astype(BF16)
    b1h = np.ascontiguousarray(b1.reshape(E, KH, 128).transpose(0, 2, 1))
    b2h = np.ascontiguousarray(b2.reshape(E, KD, 128).transpose(0, 2, 1))

    in_maps = []
    sel_packed = []
    for e in range(E):
        ne = int(n_e[e])
        idx = sel[e, :ne]
        sel_packed.append(idx)
        xT = np.zeros((128, KD, T), dtype=BF16)
        if ne:
            # [ne, D] -> [D, ne] -> [KD, 128, ne] -> [128, KD, ne]
            xT[:, :, :ne] = (
                x[idx].T.astype(BF16).reshape(KD, 128, ne).transpose(1, 0, 2)
            )
        wg = np.zeros((128, T), dtype=np.float32)
        if ne:
            wg[:, :ne] = w[idx, e][None, :]
        in_maps.append(
            {
                "xT": xT,
                "w1": w1h[e],
                "w2": w2h[e],
                "b1t": b1h[e],
                "b2t": b2h[e],
                "wg": wg,
            }
        )

    nc = _build_program(T, D, H)
    res = run_bass_kernel_spmd(nc, in_maps, core_ids=list(range(N_CORES)))

    # [E, 128, KD, T] -> [E, T, D] with d = md*128 + p
    outT = np.stack([res.results[e]["outT"] for e in range(E)])
    OUT = outT.transpose(0, 3, 2, 1).reshape(E, T, D)

    # Combine: each token sums its (up to K) expert contributions.
    final = np.zeros_like(x)
    rows = np.arange(N)
    for k in range(K):
        ek = ei[:, k]
        kept = keep[rows, ek]
        ck = np.where(kept, pos[rows, ek], 0)
        final += OUT[ek, ck] * kept[:, None].astype(np.float32)

    LAST.update(
        nc=nc,
        in_maps=in_maps,
        res=res,
        T=T,
        n_e=n_e,
        sel_packed=sel_packed,
    )
    return final, aux_loss


# revision 4
# speedup vs baseline: 7592.4539x; 7592.4539x over previous
"""MoE (top-2, 8 experts) expert-parallel kernel for 8 Trainium2 NeuronCores.

Strategy
--------
The reference pads every expert to capacity N/K = 8192 token slots, but only
~N*K/E ~ 4096 per expert carry nonzero gate weight.  The router and the
dispatch/combine bookkeeping are tiny (<0.05% of FLOPs) and routing decisions
must match the reference bit-for-bit, so they are computed with the same jax
ops on CPU (the reference itself can only run on CPU: jnp.argsort has no
trn2 lowering).  Each expert's kept tokens are packed densely and shipped to
one NeuronCore: core e holds expert e's weights in SBUF and runs
    out = (relu(xg @ W1 + b1) @ W2 + b2) * gate
in bf16 (fp32 PSUM accumulation), streaming 512-token chunks.  The host then
scatter-adds the two expert contributions per token (exact: fp addition of
two terms is commutative) and returns (final, aux_loss).
"""

import math

import numpy as np

import concourse.bass as bass
import concourse.mybir as mybir
import concourse.tile as tile
from concourse.bass_utils import run_bass_kernel_spmd

dt = mybir.dt
BF16 = mybir.dt.np(mybir.dt.bfloat16)

N_CORES = 8
CHUNK = 512

# Filled by kernel() for test harness introspection / re-benching.
LAST = {}


def _split_multiwaits(nc):
    """The walrus build in this container accepts only ONE sem-wait per
    instruction (CoreV3 setupSyncWait: 'Too many sync wait commands').
    Tile attaches several waits to its end-of-kernel drain; move every
    extra wait onto a NoOp on the same engine immediately before the
    instruction (semantically identical: all waits must pass before the
    instruction may issue)."""
    uid = 0
    for fn in nc.m.functions:
        for bb in fn.blocks:
            out = []
            changed = False
            for inst in bb.instructions:
                si = inst.sync_info
                if si is not None and len(si.on_wait) > 1:
                    waits = list(si.on_wait)
                    for w in waits[:-1]:
                        uid += 1
                        out.append(
                            mybir.InstNoOp(
                                name=f"I-mwsplit-{uid}",
                                engine=inst.engine,
                                ins=[],
                                outs=[],
                                sync_info=mybir.SyncInfo(on_wait=[w], on_update=[]),
                            )
                        )
                    si.on_wait = [waits[-1]]
                    changed = True
                out.append(inst)
            if changed:
                bb.instructions = out


_PROG_CACHE = {}


def _build_program(T, D, H, repeat=1):
    """Per-core expert MLP: xT [128, D/128, T] bf16 -> outT [128, D/128, T] f32.

    Feature-major layout keeps tokens on the free dim, so b1/b2 are
    per-partition ACT biases and the per-token gate is a replicated
    [128, T] operand for one DVE multiply.

    repeat>1 wraps the whole pipeline in a For_i loop re-processing the same
    data; used only to amplify device time for wall-clock measurement."""
    key = (T, D, H, repeat)
    if key in _PROG_CACHE:
        return _PROG_CACHE[key]

    KD = D // 128  # 8
    KH = H // 128  # 32
    # full 512-token chunks plus one 128-aligned tail chunk
    chunk_slices = []
    off = 0
    while off < T:
        size = min(CHUNK, T - off)
        chunk_slices.append((off, size))
        off += size

    nc = bass.Bass()
    xT_d = nc.dram_tensor("xT", [128, KD, T], dt.bfloat16, kind="ExternalInput")
    w1_d = nc.dram_tensor("w1", [128, KD, H], dt.bfloat16, kind="ExternalInput")
    w2_d = nc.dram_tensor("w2", [128, KH, D], dt.bfloat16, kind="ExternalInput")
    b1_d = nc.dram_tensor("b1t", [128, KH], dt.float32, kind="ExternalInput")
    b2_d = nc.dram_tensor("b2t", [128, KD], dt.float32, kind="ExternalInput")
    wg_d = nc.dram_tensor("wg", [128, T], dt.float32, kind="ExternalInput")
    out_d = nc.dram_tensor("outT", [128, KD, T], dt.float32, kind="ExternalOutput")

    with tile.TileContext(nc) as tc:
        with (
            tc.tile_pool(name="wpool", bufs=1) as wpool,
            tc.tile_pool(name="hpool", bufs=1) as hpool,
            tc.tile_pool(name="xpool", bufs=2) as xpool,
            tc.tile_pool(name="gpool", bufs=2) as gpool,
            tc.tile_pool(name="opool", bufs=3) as opool,
            tc.tile_pool(name="ps1", bufs=3, space="PSUM") as ps1,
            tc.tile_pool(name="ps2", bufs=3, space="PSUM") as ps2,
        ):
            w1_s = wpool.tile([128, KD, H], dt.bfloat16)
            for kd in range(KD):
                nc.sync.dma_start(w1_s[:, kd], w1_d[:, kd])
            w2_s = wpool.tile([128, KH, D], dt.bfloat16)
            for kh in range(0, KH, 4):
                nc.sync.dma_start(w2_s[:, kh : kh + 4], w2_d[:, kh : kh + 4])
            b1_s = wpool.tile([128, KH], dt.float32)
            nc.sync.dma_start(b1_s[:], b1_d[:])
            b2_s = wpool.tile([128, KD], dt.float32)
            nc.sync.dma_start(b2_s[:], b2_d[:])

            hT = hpool.tile([128, KH, CHUNK], dt.bfloat16)

            def body():
                for off, size in chunk_slices:
                    cs = bass.ds(off, size)
                    xc = xpool.tile([128, KD, CHUNK], dt.bfloat16)
                    nc.sync.dma_start(xc[:, :, :size], xT_d[:, :, cs])
                    gc = gpool.tile([128, CHUNK], dt.float32)
                    nc.sync.dma_start(gc[:, :size], wg_d[:, cs])

                    for mh in range(KH):
                        p1 = ps1.tile([128, CHUNK], dt.float32)
                        for kd in range(KD):
                            nc.tensor.matmul(
                                p1[:, :size],
                                w1_s[:, kd, bass.ts(mh, 128)],
                                xc[:, kd, :size],
                                start=(kd == 0),
                                stop=(kd == KD - 1),
                            )
                        nc.scalar.activation(
                            hT[:, mh, :size],
                            p1[:, :size],
                            mybir.ActivationFunctionType.Relu,
                            bias=b1_s[:, mh : mh + 1],
                        )

                    for md in range(KD):
                        p2 = ps2.tile([128, CHUNK], dt.float32)
                        for kh in range(KH):
                            nc.tensor.matmul(
                                p2[:, :size],
                                w2_s[:, kh, bass.ts(md, 128)],
                                hT[:, kh, :size],
                                start=(kh == 0),
                                stop=(kh == KH - 1),
                            )
                        ot = opool.tile([128, CHUNK], dt.float32)
                        nc.scalar.activation(
                            ot[:, :size],
                            p2[:, :size],
                            mybir.ActivationFunctionType.Identity,
                            bias=b2_s[:, md : md + 1],
                        )
                        nc.vector.tensor_mul(
                            out=ot[:, :size], in0=ot[:, :size], in1=gc[:, :size]
                        )
                        nc.sync.dma_start(out_d[:, md, cs], ot[:, :size])

            if repeat > 1:
                with tc.For_i(0, repeat, 1):
                    body()
            else:
                body()

    _split_multiwaits(nc)
    _PROG_CACHE[key] = nc
    return nc


def _route(x, Wr, br, E, K):
    """Replicate the reference router bit-for-bit with the same jax ops on
    CPU (the backend the reference itself must run on)."""
    import jax
    import jax.numpy as jnp

    n_tok = x.shape[0]
    capacity = max(int(1.0 * n_tok / K), 1)  # CAPACITY_FACTOR=1, WORLD_SIZE=1

    cpu = jax.devices("cpu")[0]
    with jax.default_device(cpu):
        xj = jnp.asarray(x)
        Wrj = jnp.asarray(Wr)
        brj = jnp.asarray(br)

        logits = xj @ Wrj + brj
        probs = jax.nn.softmax(logits, axis=-1)
        ew, ei = jax.lax.top_k(probs, K)

        match = ei[:, :, None] == jnp.arange(E)[None, None, :]
        mask = match.any(axis=1)
        w = (ew[:, :, None] * match).sum(axis=1)

        pos = jnp.cumsum(mask.astype(jnp.int32), axis=0) - 1
        keep = mask & (pos < capacity)
        keepT = keep.T
        sel = jax.vmap(lambda k: jnp.argsort(jnp.logical_not(k))[:capacity])(keepT)

        expert_counts = mask.astype(probs.dtype).sum(axis=0)
        density = probs.mean(axis=0)
        usage = expert_counts / n_tok
        balance_loss = (density * usage).sum() * E
        importance = probs.sum(axis=0)
        importance_loss = (importance**2).mean()
        aux_loss = balance_loss + importance_loss

    return (
        np.asarray(sel),
        np.asarray(keep),
        np.asarray(pos),
        np.asarray(ei),
        np.asarray(w),
        np.asarray(aux_loss),
    )


def kernel(x, Wr, br, W1, b1, W2, b2):
    x = np.asarray(x)
    Wr = np.asarray(Wr)
    br = np.asarray(br)
    W1 = np.asarray(W1)
    b1 = np.asarray(b1)
    W2 = np.asarray(W2)
    b2 = np.asarray(b2)

    N, D = x.shape
    E, _, H = W1.shape
    K = 2

    sel, keep, pos, ei, w, aux_loss = _route(x, Wr, br, E, K)

    n_e = keep.sum(axis=0).astype(np.int64)  # kept tokens per expert
    T = max(128, int(math.ceil(n_e.max() / 128)) * 128)

    KD = D // 128
    KH = H // 128

    # Pack per-expert inputs (feature-major, bf16) + replicated gate rows.
    w1h = np.ascontiguousarray(
        W1.reshape(E, KD, 128, H).transpose(0, 2, 1, 3)
    ).astype(BF16)
    w2h = np.ascontiguousarray(
        W2.reshape(E, KH, 128, D).transpose(0, 2, 1, 3)
    ).astype(BF16)
    b1h = np.ascontiguousarray(b1.reshape(E, KH, 128).transpose(0, 2, 1))
    b2h = np.ascontiguousarray(b2.reshape(E, KD, 128).transpose(0, 2, 1))

    in_maps = []
    sel_packed = []
    for e in range(E):
        ne = int(n_e[e])
        idx = sel[e, :ne]
        sel_packed.append(idx)
        xT = np.zeros((128, KD, T), dtype=BF16)
        if ne:
            # [ne, D] -> [D, ne] -> [KD, 128, ne] -> [128, KD, ne]
            xT[:, :, :ne] = (
                x[idx].T.astype(BF16).reshape(KD, 128, ne).transpose(1, 0, 2)
            )
        wg = np.zeros((128, T), dtype=np.float32)
        if ne:
            wg[:, :ne] = w[idx, e][None, :]
        in_maps.append(
            {
                "xT": xT,
                "w1": w1h[e],
                "w2": w2h[e],
                "b1t": b1h[e],
                "b2t": b2h[e],
                "wg": wg,
            }
        )

    nc = _build_program(T, D, H)
    res = run_bass_kernel_spmd(nc, in_maps, core_ids=list(range(N_CORES)))

    # [E, 128, KD, T] -> [E, T, D] with d = md*128 + p
    outT = np.stack([res.results[e]["outT"] for e in range(E)])
    OUT = outT.transpose(0, 3, 2, 1).reshape(E, T, D)

    # Combine: each token sums its (up to K) expert contributions.
    final = np.zeros_like(x)
    rows = np.arange(N)
    for k in range(K):
        ek = ei[:, k]
        kept = keep[rows, ek]
        ck = np.where(kept, pos[rows, ek], 0)
        final += OUT[ek, ck] * kept[:, None].astype(np.float32)

    LAST.update(
        nc=nc,
        in_maps=in_maps,
        res=res,
        T=T,
        n_e=n_e,
        sel_packed=sel_packed,
    )
    return final, aux_loss
